# revision 2
# baseline (speedup 1.0000x reference)
"""MoE routing gate kernel for Trainium2 (8 NeuronCores, data-parallel).

Computes, for x[32768, 2048], weight[64, 2048], bias[64]:
    logits = x @ weight.T
    probs  = softmax(logits, axis=-1)
    idx    = top_k(probs + bias, 6).indices
    w      = take_along_axis(probs, idx)
returning (w float32 [32768, 6], idx int32 [32768, 6]).

Sharding: tokens split 4096/core across 8 cores; weight/bias replicated.

Per-core pipeline (memory-bound; HBM floor ~67us for the 24.6MB shard):
  - x is streamed at 3 bytes/element: fp16 hi + fp8e4m3 lo with
    lo = (x - fp16(x)) * 2048. Three matmul passes accumulate fp32
    logits in PSUM: hi @ w_hi(fp16) + hi @ w_lo(fp16) + lo8 @ w3(fp16)
    giving |logit err| ~2.5e-5 (needed: top-k index flips explode below
    ~16 mantissa bits because biased-score gaps at the rank-6 boundary
    are ~1e-4).
  - Matmul pairs are column-tiled: group g=0 lands in PE columns 0-63,
    g=1 in columns 64-127, so two half-width matmuls stream concurrently
    through disjoint column groups of the array.
  - Super-groups: 3x1024 tokens + 2x512. The small trailing groups
    shrink the serial tail (compute exposed after the last HBM byte).
    The 512-token groups' chunks live in dedicated, fully-buffered SBUF
    pools so their DMA issue is never gated on the PE freeing buffers.
  - Per super-group finish: logits^T -> ACT copy -> PE transposes into
    two PSUM banks (bases 0/64 must not share a bank), per-j ACT exp
    (accum_out = row sum), DVE q = exp + sum*bias (ranks identically to
    probs + bias), Max8/MaxIndex8.
  - Output is packed per 128-token tile into one [128, nj, 15] f32 tile:
    cols 0-7 top-8 q values, 8-13 top-6 indices (u32->f32 cast, exact
    for idx<64), 14 row sum of exp. ~250KB/core instead of shipping all
    64 exp values (1.1MB): the host reconstructs
    w_k = (q_k - sum*bias[idx_k]) / sum.
"""

import numpy as np
import ml_dtypes

import concourse.bacc as bacc
import concourse.bass as bass
import concourse.mybir as mybir
import concourse.tile as tile
from concourse.bass_utils import run_bass_kernel_spmd

F32 = mybir.dt.float32
F16 = mybir.dt.float16
F8E4 = mybir.dt.float8e4
I32 = mybir.dt.int32
U32 = mybir.dt.uint32
AX = mybir.AxisListType
OP = mybir.AluOpType
EXP = mybir.ActivationFunctionType.Exp

TOKENS, DIM, E, TOPK, NCORES = 32768, 2048, 64, 6, 8
KC = DIM // 128          # contraction chunks of 128
SGS_A = 3                # super-groups of 1024 tokens
SGT_A, KQ_A = 1024, 4    # chunk = [128, KQ, sgt]
SGS_B = 2                # trailing super-groups of 512 tokens
SGT_B, KQ_B = 512, 4
PACKW = 15               # mx8 | mi6 | sum


def build_nc():
    nc = bacc.Bacc("TRN2", target_bir_lowering=False, debug=False)

    xhi_a = nc.dram_tensor(
        "xhi_a", [SGS_A, KC // KQ_A, 128, KQ_A, SGT_A], F16, kind="ExternalInput"
    )
    xlo_a = nc.dram_tensor(
        "xlo_a", [SGS_A, KC // KQ_A, 128, KQ_A, SGT_A], F8E4, kind="ExternalInput"
    )
    xhi_b = nc.dram_tensor(
        "xhi_b", [SGS_B, KC // KQ_B, 128, KQ_B, SGT_B], F16, kind="ExternalInput"
    )
    xlo_b = nc.dram_tensor(
        "xlo_b", [SGS_B, KC // KQ_B, 128, KQ_B, SGT_B], F8E4, kind="ExternalInput"
    )
    wt_hi = nc.dram_tensor("wt_hi", [128, KC, E], F16, kind="ExternalInput")
    wt_lo = nc.dram_tensor("wt_lo", [128, KC, E], F16, kind="ExternalInput")
    wt_3 = nc.dram_tensor("wt_3", [128, KC, E], F16, kind="ExternalInput")
    bias_b = nc.dram_tensor("bias_b", [128, E], F32, kind="ExternalInput")
    ident2 = nc.dram_tensor("ident2", [128, 64], F32, kind="ExternalInput")
    o_pk_a = nc.dram_tensor(
        "o_pk_a", [SGS_A, 128, SGT_A // 128, PACKW], F32, kind="ExternalOutput"
    )
    o_pk_b = nc.dram_tensor(
        "o_pk_b", [SGS_B, 128, SGT_B // 128, PACKW], F32, kind="ExternalOutput"
    )

    with tile.TileContext(nc) as tc:
        with (
            tc.tile_pool(name="consts", bufs=1) as cpool,
            tc.tile_pool(name="xha", bufs=6) as xhap,
            tc.tile_pool(name="xla", bufs=6) as xlap,
            tc.tile_pool(name="xhb", bufs=4) as xhbp,
            tc.tile_pool(name="xlb", bufs=4) as xlbp,
            tc.tile_pool(name="lt", bufs=3) as ltp,
            tc.tile_pool(name="ex", bufs=3) as exp_,
            tc.tile_pool(name="wk", bufs=2) as wkp,
            tc.tile_pool(name="small", bufs=3) as smp,
            tc.tile_pool(name="acc", bufs=3, space="PSUM") as accp,
            tc.tile_pool(name="tr", bufs=2, space="PSUM") as trp,
        ):
            cwh = cpool.tile([128, KC, E], F16)
            nc.scalar.dma_start(cwh, wt_hi[:])
            cwl = cpool.tile([128, KC, E], F16)
            nc.scalar.dma_start(cwl, wt_lo[:])
            cw3 = cpool.tile([128, KC, E], F16)
            nc.scalar.dma_start(cw3, wt_3[:])
            cbias = cpool.tile([128, E], F32)
            nc.scalar.dma_start(cbias, bias_b[:])
            cident = cpool.tile([128, 64], F32)
            nc.scalar.dma_start(cident, ident2[:])

            def finish_sg(out_dram, idx, acc, sgt):
                """Transpose/softmax/rank/pack for a finished super-group.

                Deferred one sg behind the matmul issue so the Tensor queue
                always has the next sg's matmuls ahead of these transposes
                (which wait on the ACT copy) - avoids a cross-engine convoy.
                """
                nj = sgt // 128
                half = nj // 2
                grp = sgt // 2

                lt = ltp.tile([128, 512], F32, tag="lt")
                nc.scalar.copy(lt[0:64, 0:grp], acc[0:64, 0:grp])
                nc.scalar.copy(lt[64:128, 0:grp], acc[64:128, 0:grp])

                # transposes into two PSUM tiles; tiles are padded to a full
                # 2KB bank so base-0 and base-64 reads never share a bank
                # (sharing hangs the HW).
                tpsA = trp.tile([128, 8, E], F32, tag="tpsA")
                tpsB = trp.tile([128, 8, E], F32, tag="tpsB")
                for j in range(nj):
                    base = 64 * (j // half)
                    tps = tpsA if j < half else tpsB
                    nc.tensor.transpose(
                        tps[:, j % half],
                        lt[base:base + 64, (j % half) * 128:(j % half + 1) * 128],
                        cident[base:base + 64, :],
                    )

                ex = exp_.tile([128, 8, E], F32, tag="ex")
                q = wkp.tile([128, 8, E], F32, tag="q")
                pk = smp.tile([128, 8, 16], F32, tag="pk")
                mi = smp.tile([128, 8, 8], U32, tag="mi")
                for j in range(nj):
                    tps = (tpsA if j < half else tpsB)[:, j % half]
                    nc.scalar.activation(
                        ex[:, j], tps, EXP, accum_out=pk[:, j, 14:15]
                    )
                    nc.vector.scalar_tensor_tensor(
                        q[:, j], cbias, pk[:, j, 14:15], ex[:, j],
                        OP.mult, OP.add,
                    )
                    nc.vector.max(pk[:, j, 0:8], q[:, j])
                    nc.vector.max_index(mi[:, j], pk[:, j, 0:8], q[:, j])
                    # u32 -> f32 value cast; exact for idx < 64
                    nc.vector.tensor_copy(pk[:, j, 8:14], mi[:, j, 0:TOPK])

                nc.gpsimd.dma_start(out_dram[idx], pk[:, 0:nj, 0:PACKW])

            # (dram, idx-in-group, sgt, KQ, hi dram, lo dram, hi pool, lo pool)
            sched = [
                (o_pk_a, i, SGT_A, KQ_A, xhi_a, xlo_a, xhap, xlap)
                for i in range(SGS_A)
            ] + [
                (o_pk_b, i, SGT_B, KQ_B, xhi_b, xlo_b, xhbp, xlbp)
                for i in range(SGS_B)
            ]

            pending = None  # (out_dram, idx, acc, sgt) awaiting finish
            for out_dram, idx, sgt, kq, xhi, xlo, hp, lp in sched:
                grp = sgt // 2
                xh, xl = [], []
                for c in range(KC // kq):
                    th = hp.tile([128, kq, sgt], F16, tag="xh")
                    nc.sync.dma_start(th, xhi[idx, c])
                    xh.append(th)
                    tl = lp.tile([128, kq, sgt], F8E4, tag="xl")
                    # lo rides the sync ring right behind its hi chunk:
                    # queue order matches consumption order exactly
                    nc.sync.dma_start(tl, xlo[idx, c])
                    xl.append(tl)

                # col-tiled matmul pairs (g=0 -> cols 0-63, g=1 -> 64-127).
                # The previous sg's finish-phase instructions are issued
                # after this sg's first chunk so the in-order Tensor queue
                # runs its transposes inside a DMA-wait bubble instead of
                # stalling the matmul stream.
                acc = accp.tile([128, 512], F32)
                for k in range(KC):
                    hi_k = xh[k // kq][:, k % kq]   # [128, sgt] fp16
                    lo_k = xl[k // kq][:, k % kq]   # [128, sgt] fp8
                    for p in range(3):
                        w = (cwh, cwl, cw3)[p][:, k, :]
                        xs = (hi_k, hi_k, lo_k)[p]
                        first = k == 0 and p == 0
                        last = k == KC - 1 and p == 2
                        nc.tensor.matmul(
                            acc[0:64, 0:grp], w, xs[:, 0:grp],
                            start=first, stop=last, tile_position=(0, 0),
                        )
                        nc.tensor.matmul(
                            acc[64:128, 0:grp], w, xs[:, grp:sgt],
                            start=first, stop=last, tile_position=(0, 64),
                            skip_group_check=True,
                        )
                    if k == kq - 1 and pending is not None:
                        finish_sg(*pending)
                        pending = None
                pending = (out_dram, idx, acc, sgt)
            finish_sg(*pending)
    return nc


_CACHE = {}


def _get_compiled():
    if "nc" not in _CACHE:
        nc = build_nc()
        nc.compile()
        _CACHE["nc"] = nc
    return _CACHE["nc"]


def _prep_shared(weight, bias):
    f16 = np.float16
    w = np.asarray(weight, np.float32)
    w_hi = w.astype(f16)
    w_lo = (w - w_hi.astype(np.float32)).astype(f16)
    w_3 = (w_hi.astype(np.float32) * (1.0 / 2048.0)).astype(f16)

    def wtile(a):  # [E, DIM] -> [128, KC, E]
        return np.ascontiguousarray(
            np.ascontiguousarray(a.T).reshape(KC, 128, E).transpose(1, 0, 2)
        )

    return {
        "wt_hi": wtile(w_hi),
        "wt_lo": wtile(w_lo),
        "wt_3": wtile(w_3),
        "bias_b": np.ascontiguousarray(
            np.broadcast_to(np.asarray(bias, np.float32), (128, E))
        ),
        "ident2": np.ascontiguousarray(
            np.tile(np.eye(64, dtype=np.float32), (2, 1))
        ),
    }


def _pack_group(xT, nsg, sgt, kq):
    # [DIM, nsg*sgt] -> [nsg, KC//kq, 128, kq, sgt]; per (sg, chunk,
    # partition) the [kq, sgt] block is one contiguous run in DRAM
    x6 = xT.reshape(KC // kq, kq, 128, nsg, sgt)
    return np.ascontiguousarray(x6.transpose(3, 0, 2, 1, 4))


def prep_core_inputs(x, weight, bias, ncores=NCORES):
    shared = _prep_shared(weight, bias)
    x = np.asarray(x, np.float32)
    tpc = x.shape[0] // ncores
    na = SGS_A * SGT_A
    # whole-tensor transpose + casts once (not per core)
    xT = np.ascontiguousarray(x.T)           # [DIM, TOKENS]
    xhT = xT.astype(np.float16)
    xlT = ((xT - xhT.astype(np.float32)) * 2048.0).astype(
        ml_dtypes.float8_e4m3fn
    )
    del xT
    in_maps = []
    for c in range(ncores):
        lo = c * tpc
        in_maps.append({
            "xhi_a": _pack_group(xhT[:, lo:lo + na], SGS_A, SGT_A, KQ_A),
            "xlo_a": _pack_group(xlT[:, lo:lo + na], SGS_A, SGT_A, KQ_A),
            "xhi_b": _pack_group(
                xhT[:, lo + na:lo + tpc], SGS_B, SGT_B, KQ_B
            ),
            "xlo_b": _pack_group(
                xlT[:, lo + na:lo + tpc], SGS_B, SGT_B, KQ_B
            ),
            **shared,
        })
    return in_maps


def unpack_outputs(res_list, bias):
    bias = np.asarray(bias, np.float64)
    ws, idxs = [], []
    for r in res_list:
        for nm in ("o_pk_a", "o_pk_b"):
            pk = np.asarray(r[nm], np.float64)  # [nsg, 128, nj, 15]
            # token t = sg*sgt + 128*j + p
            pk = pk.transpose(0, 2, 1, 3).reshape(-1, PACKW)
            mx = pk[:, 0:TOPK]
            mi = np.rint(pk[:, 8:14]).astype(np.int64)
            ssum = pk[:, 14:15]
            wv = (mx - ssum * bias[mi]) / ssum
            ws.append(wv)
            idxs.append(mi)
    return (
        np.ascontiguousarray(np.concatenate(ws)).astype(np.float32),
        np.ascontiguousarray(np.concatenate(idxs)).astype(np.int32),
    )


def run(x, weight, bias, trace=False, **kwargs):
    x = np.asarray(x, np.float32)
    nc = _get_compiled()
    in_maps = prep_core_inputs(x, weight, bias)
    res = run_bass_kernel_spmd(
        nc, in_maps, list(range(NCORES)), trace=trace, **kwargs
    )
    w, i = unpack_outputs(res.results, bias)
    return w, i, res


def kernel(x, weight, bias):
    w, i, _ = run(x, weight, bias, trace=False)
    return w, i
